# revision 16
# baseline (speedup 1.0000x reference)
"""MoE (top-2 of 8 experts, SwiGLU) Trainium2 kernel — balanced 2-slot version.

Sharding strategy (expert-parallel with load balancing):
  - Host computes the gate (tiny [T,8] matmul), top-2 routing and softmax
    weights. Tokens are grouped by expert; each expert's token list is split
    across up to two fixed-capacity "slots". Every core runs the SAME program
    with two slots (capacities cA, cB chosen from the routing): each slot has
    its own expert weight set (W1/W3/W2) and token block, so per-core work is
    identical (cA+cB streamed columns) regardless of expert load imbalance.
  - Core: per slot, y = gate_w * (silu(x @ W1e.T) * (x @ W3e.T)) @ W2e.T in
    feature-major layout (features on partitions, tokens on the free axis).
  - Host scatter-adds each slot's output rows back into the full output.

Matmuls run in fp16 (full PE rate, fp32 PSUM accumulation). W1/W3 weight
streams, the gate vector and the y output ride the sync HWDGE queue; x and W2
ride the scalar HWDGE queue, so the PE is never starved during the ramp.
"""

import numpy as np

import concourse.bass as bass
import concourse.mybir as mybir
from concourse import bacc
from concourse import tile
from concourse.bass_utils import run_bass_kernel_spmd

DIM = 1024
HID = 2816
E = 8
TOPK = 2
P = 128
KD = DIM // P  # 8 k-tiles over DIM
KH = HID // P  # 22 k-tiles over HID
F32 = mybir.dt.float32
import os as _os
_MM_DT_NAME = _os.environ.get("KERNEL_MM_DT", "float16")
MM_DT = getattr(mybir.dt, _MM_DT_NAME)
_NP_MM = {"float32r": np.float32, "float32": np.float32}.get(_MM_DT_NAME)
if _NP_MM is None:
    import ml_dtypes as _mld
    _NP_MM = {"float16": np.float16, "bfloat16": _mld.bfloat16}[_MM_DT_NAME]
PREFETCH_W = 5  # weight h-tile pairs in flight (= wload bufs)

# Test hooks: when TRACE is set (by test.py), the SPMD launch captures an
# NTFF profile and the BassKernelResults lands in LAST_RESULTS.
TRACE = False
LAST_RESULTS = None

_nc_cache: dict = {}


def _halves(c):
    """Slot slicing: one slice if it fits a PSUM bank (512 fp32) — pipelining
    then comes from bufs=2 on the PSUM tag — else two halves."""
    if c <= 512:
        return [(0, c)]
    h = (c + 1) // 2
    return [(0, h), (h, c - h)]


def _build_nc(cA, cB):
    """Per-core Bass program: two expert slots of capacities cA and cB."""
    nc = bacc.Bacc(
        "TRN2",
        target_bir_lowering=False,
        debug=False,
        enable_asserts=False,
        num_devices=E,
    )

    C = cA + cB
    slot_caps = [cA, cB]
    slot_offs = [0, cA]
    slot_slices = [_halves(cA), _halves(cB)]
    # x regions: one contiguous [P, KD*tn] block per (slot, slice) so each is
    # a single DMA with multi-KB per-partition lines (fast, order-robust).
    x_offs = {}
    run = 0
    for s in range(2):
        for t0, tn in slot_slices[s]:
            x_offs[(s, t0)] = run
            run += KD * tn
    assert run == KD * C

    xt_d = nc.dram_tensor("xp", [P, KD * C], MM_DT, kind="ExternalInput").ap()
    w13_d = {}
    w2_d = {}
    for s, tag in enumerate("ab"):
        w13_d[s] = (
            nc.dram_tensor(f"w1{tag}", [KH, P, KD * P], MM_DT, kind="ExternalInput").ap(),
            nc.dram_tensor(f"w3{tag}", [KH, P, KD * P], MM_DT, kind="ExternalInput").ap(),
        )
        w2_d[s] = nc.dram_tensor(f"w2{tag}", [KD, P, KH * P], MM_DT, kind="ExternalInput").ap()
    gw_d = nc.dram_tensor("gwp", [P, C], F32, kind="ExternalInput").ap()
    yt_d = nc.dram_tensor("yt", [KD, P, C], MM_DT, kind="ExternalOutput").ap()

    with tile.TileContext(nc) as tc:
        with (
            tc.tile_pool(name="xpool", bufs=1) as xpool,
            tc.tile_pool(name="wload", bufs=PREFETCH_W) as wload,
            tc.tile_pool(name="w2load", bufs=3) as w2load,
            tc.tile_pool(name="gpool", bufs=1) as gpool,
            tc.tile_pool(name="spool", bufs=2) as spool,
            tc.tile_pool(name="ypool", bufs=3) as ypool,
            tc.tile_pool(name="psA", bufs=1, space="PSUM") as psApool,
            tc.tile_pool(name="psB", bufs=1, space="PSUM") as psBpool,
        ):
            # PE pre-warm: dummy matmuls on a zeroed tile keep the PE busy
            # (and flip the HAM clock gate to 8/8) while input DMAs stream.
            t_warm = xpool.tile([P, 512], mybir.dt.bfloat16, tag="warm")
            nc.vector.memset(t_warm, 0.0)
            ps_warm = psApool.tile([P, 512], F32, tag="ps_s0_0")
            for _ in range(11):
                nc.tensor.matmul(
                    ps_warm, lhsT=t_warm[:, :P], rhs=t_warm, start=True, stop=True
                )

            from collections import deque

            # ---- weight h-tile stream (sync queue, consumption order) ----
            w1_tiles: deque = deque()
            w3_tiles: deque = deque()
            wq = [(s, i) for s in range(2) for i in range(KH)]
            wq_pos = [0]

            def pump_w13():
                if wq_pos[0] >= len(wq):
                    return
                s, i = wq[wq_pos[0]]
                wq_pos[0] += 1
                t1 = wload.tile([P, KD * P], MM_DT, tag="w1", name=f"w1_{s}_{i}")
                nc.sync.dma_start(out=t1, in_=w13_d[s][0][i])
                w1_tiles.append(t1)
                t3 = wload.tile([P, KD * P], MM_DT, tag="w3", name=f"w3_{s}_{i}")
                nc.sync.dma_start(out=t3, in_=w13_d[s][1][i])
                w3_tiles.append(t3)

            # ---- w2 d-tile stream (scalar queue) ----
            w2_tiles: deque = deque()
            w2q = [(s, d) for s in range(2) for d in range(KD)]
            w2q_pos = [0]

            def pump_w2():
                if w2q_pos[0] >= len(w2q):
                    return
                s, dd = w2q[w2q_pos[0]]
                w2q_pos[0] += 1
                t2 = w2load.tile([P, KH * P], MM_DT, tag="w2", name=f"w2_{s}_{dd}")
                nc.scalar.dma_start(out=t2, in_=w2_d[s][dd])
                w2_tiles.append(t2)

            # Activations on the scalar queue (parallel with the sync-queue
            # weight stream): one contiguous DMA per (slot, slice), slot-a
            # slice-0 first — it gates the first real matmul.
            t_x = {}
            for s in range(2):
                for t0, tn in slot_slices[s]:
                    o = x_offs[(s, t0)]
                    t = xpool.tile(
                        [P, KD * tn], MM_DT, tag=f"x{s}_{t0}", name=f"x_{s}_{t0}"
                    )
                    if s == 0 and t0 == 0:
                        # first slice in two halves so the first k-tiles land
                        # (and unblock the first matmul group) sooner
                        h = (KD // 2) * tn
                        nc.scalar.dma_start(out=t[:, :h], in_=xt_d[:, o : o + h])
                        nc.scalar.dma_start(
                            out=t[:, h:], in_=xt_d[:, o + h : o + KD * tn]
                        )
                    else:
                        nc.scalar.dma_start(out=t, in_=xt_d[:, o : o + KD * tn])
                    t_x[(s, t0)] = t

            for _ in range(PREFETCH_W):
                pump_w13()  # (a, 0..4): (a, 0) is the ramp critical path
            t_gw = xpool.tile([P, C], F32, tag="gw")

            g_tiles = {0: [], 1: []}

            def stage1(s):
                cap = slot_caps[s]
                slices = slot_slices[s]
                for i in range(KH):
                    pump_w13()
                    if s == 0 and i == 2:
                        # gate weights ride the sync queue mid-stream; they are
                        # only needed by stage 2.
                        nc.sync.dma_start(out=t_gw, in_=gw_d)
                    if i == KH - 6:
                        pump_w2()  # this slot's first two w2 d-tiles load
                    if i == KH - 3:
                        pump_w2()  # during the stage-1 tail
                    t_w1 = w1_tiles.popleft()
                    t_w3 = w3_tiles.popleft()

                    t_g = gpool.tile([P, cap], MM_DT, tag=f"g{s}_{i}")
                    g_tiles[s].append(t_g)

                    nslice = len(slices)
                    ps1s, ps3s = [], []
                    for si, (t0, tn) in enumerate(slices):
                        ps1s.append(psApool.tile(
                            [P, 512], F32, tag=f"ps_s{s}_{si}",
                            bufs=(2 if nslice == 1 else 1), name=f"ps1_{s}_{i}_{t0}"
                        ))
                        ps3s.append(psBpool.tile(
                            [P, 512], F32, tag=f"ps_s{s}_{si}",
                            bufs=(2 if nslice == 1 else 1), name=f"ps3_{s}_{i}_{t0}"
                        ))
                    # Slices interleaved inside the k-loop: back-to-back
                    # matmuls share each weight tile (536 streamed cols per
                    # load instead of 268). The first two h-tiles keep the
                    # sequential order so the ramp only waits on slice 0's x.
                    if s == 0 and i < 2 and nslice > 1:
                        slice_phases = [[si] for si in range(nslice)]
                    else:
                        slice_phases = [list(range(nslice))]
                    for w_tile, ps_grp in ((t_w1, ps1s), (t_w3, ps3s)):
                        for phase in slice_phases:
                            for k in range(KD):
                                for si in phase:
                                    t0, tn = slices[si]
                                    nc.tensor.matmul(
                                        ps_grp[si][:, :tn],
                                        lhsT=w_tile[:, k * P : (k + 1) * P],
                                        rhs=t_x[(s, t0)][:, k * tn : (k + 1) * tn],
                                        start=(k == 0),
                                        stop=(k == KD - 1),
                                    )
                    for si, (t0, tn) in enumerate(slices):
                        t_sg = spool.tile([P, 512], F32, tag="sig")
                        nc.scalar.activation(
                            t_sg[:, :tn],
                            ps1s[si][:, :tn],
                            mybir.ActivationFunctionType.Sigmoid,
                        )
                        t_s = spool.tile([P, 512], F32, tag="silu")
                        nc.vector.tensor_mul(t_s[:, :tn], t_sg[:, :tn], ps1s[si][:, :tn])
                        nc.vector.tensor_mul(
                            t_g[:, t0 : t0 + tn], t_s[:, :tn], ps3s[si][:, :tn]
                        )

            def stage2(s):
                off = slot_offs[s]
                for dt_i in range(KD):
                    slices = slot_slices[s]
                    if s == 1 and dt_i == KD - 1 and len(slices) == 1:
                        # split the very last d-tile so the final mul + y DMA
                        # chain is half-size and overlaps the preceding MMs
                        c = slot_caps[s]
                        h = (c + 1) // 2
                        slices = [(0, h), (h, c - h)]
                    pump_w2()
                    t_w2 = w2_tiles.popleft()
                    nslice = len(slot_slices[s])
                    for si, (t0, tn) in enumerate(slices):
                        # a split-in-two last d-tile reuses tag 0 (bufs=2 ring)
                        tag_si = min(si, nslice - 1)
                        psy = psApool.tile(
                            [P, 512], F32, tag=f"ps_s{s}_{tag_si}",
                            bufs=(2 if nslice == 1 else 1), name=f"psy_{s}_{dt_i}_{t0}"
                        )
                        for i in range(KH):
                            nc.tensor.matmul(
                                psy[:, :tn],
                                lhsT=t_w2[:, i * P : (i + 1) * P],
                                rhs=g_tiles[s][i][:, t0 : t0 + tn],
                                start=(i == 0),
                                stop=(i == KH - 1),
                            )
                        t_y = ypool.tile([P, 512], MM_DT, tag="y")
                        nc.vector.tensor_mul(
                            t_y[:, :tn], psy[:, :tn], t_gw[:, off + t0 : off + t0 + tn]
                        )
                        nc.sync.dma_start(
                            out=yt_d[dt_i][:, off + t0 : off + t0 + tn],
                            in_=t_y[:, :tn],
                        )

            stage1(0)
            stage2(0)
            stage1(1)
            stage2(1)

    nc.compile()
    return nc


def _route(xt, Wg):
    """Top-2 routing identical to the reference (argmax twice + softmax)."""
    scores = xt @ Wg.T  # [T, E] fp32
    top1 = np.argmax(scores, axis=1)
    v1 = scores[np.arange(scores.shape[0]), top1]
    masked = scores.copy()
    masked[np.arange(scores.shape[0]), top1] = -np.inf
    top2 = np.argmax(masked, axis=1)
    v2 = masked[np.arange(scores.shape[0]), top2]
    # softmax over [v1, v2] in fp32 (v1 >= v2)
    e2 = np.exp((v2 - v1).astype(np.float32))
    w1 = (1.0 / (1.0 + e2)).astype(np.float32)
    w2 = (e2 / (1.0 + e2)).astype(np.float32)
    return top1, top2, w1, w2


def _plan_slots(counts):
    """Choose slot capacities (cA, cB) and assign each expert's tokens to
    A/B slots (8 of each, one per core) so every expert fits and cA+cB is
    minimal. Returns (cA, cB, slotsA, slotsB) with slots* = [(e, start, len)].
    """
    counts = np.asarray(counts, dtype=np.int64)
    order = np.argsort(-counts, kind="stable")
    n = counts[order]
    total = int(counts.sum())
    mean_cap = -(-total // E)
    best = None
    for k in range(0, E // 2 + 1):
        cA_min = -(-int(n[0]) // 2) if k > 0 else 0
        cB_min = -(-int(n[E - k]) // 2) if k > 0 else 0
        mid = n[k : E - k]
        mid_max = int(mid[0]) if len(mid) else 0
        Ck = max(cA_min + cB_min, mid_max, mean_cap)
        if best is None or Ck < best[0]:
            best = (Ck, k)
    Ck, k = best
    if k > 0:
        cB = -(-int(n[E - k]) // 2)
        cA = Ck - cB
    else:
        cA = (Ck + 1) // 2
        cB = Ck - cA
    # degenerate-routing guard: keep both slots non-trivial
    cA, cB = max(cA, 16), max(cB, 16)
    slotsA, slotsB = [], []
    for i, e in enumerate(order):
        ne = int(counts[e])
        if k > 0 and i < k:  # two A slots
            a1 = min(cA, ne)
            slotsA.append((e, 0, a1))
            slotsA.append((e, a1, ne - a1))
        elif i >= E - k:  # two B slots
            b1 = min(cB, ne)
            slotsB.append((e, 0, b1))
            slotsB.append((e, b1, ne - b1))
        else:  # one A + one B slot
            a1 = min(cA, ne)
            slotsA.append((e, 0, a1))
            slotsB.append((e, a1, ne - a1))
    assert len(slotsA) == E and len(slotsB) == E
    assert all(l <= cA for _, _, l in slotsA)
    assert all(l <= cB for _, _, l in slotsB)
    return cA, cB, slotsA, slotsB


def _pack_weights(W1, W3, W2, e):
    # w1p[i, p, k, c] = W1T[k*P+p, i*P+c] = W1[e, i*P+c, k*P+p]
    w1p = np.ascontiguousarray(
        W1[e].reshape(KH, P, KD, P).transpose(0, 3, 2, 1).astype(_NP_MM)
    ).reshape(KH, P, KD * P)
    w3p = np.ascontiguousarray(
        W3[e].reshape(KH, P, KD, P).transpose(0, 3, 2, 1).astype(_NP_MM)
    ).reshape(KH, P, KD * P)
    # w2p[dt, p, i, c] = W2T[i*P+p, dt*P+c] = W2[e, dt*P+c, i*P+p]
    w2p = np.ascontiguousarray(
        W2[e].reshape(KD, P, KH, P).transpose(0, 3, 2, 1).astype(_NP_MM)
    ).reshape(KD, P, KH * P)
    return w1p, w3p, w2p


def kernel(x, Wg, W1, W3, W2):
    x = np.asarray(x, dtype=np.float32)
    Wg = np.asarray(Wg, dtype=np.float32)
    W1 = np.asarray(W1, dtype=np.float32)
    W3 = np.asarray(W3, dtype=np.float32)
    W2 = np.asarray(W2, dtype=np.float32)

    Bsz, Ssz, _ = x.shape
    T = Bsz * Ssz
    xt = x.reshape(T, DIM)

    top1, top2, wt1, wt2 = _route(xt, Wg)

    idx_lists = []
    gw_lists = []
    counts = np.zeros(E, dtype=np.int64)
    for e in range(E):
        m1 = np.nonzero(top1 == e)[0]
        m2 = np.nonzero(top2 == e)[0]
        idx_lists.append(np.concatenate([m1, m2]))
        gw_lists.append(np.concatenate([wt1[m1], wt2[m2]]))
        counts[e] = len(idx_lists[e])

    cA, cB, slotsA, slotsB = _plan_slots(counts)
    C = cA + cB

    if (cA, cB) not in _nc_cache:
        _nc_cache[(cA, cB)] = _build_nc(cA, cB)
    nc = _nc_cache[(cA, cB)]

    packs = {e: None for e in range(E)}

    def pack(e):
        if packs[e] is None:
            packs[e] = _pack_weights(W1, W3, W2, e)
        return packs[e]

    xt_mm = xt.T.astype(_NP_MM)  # [DIM, T] once

    # x region layout must match _build_nc: [(slot, slice)] contiguous blocks
    # of [P, KD*tn], per-partition line = the KD k-tiles of that slice.
    def _pack_x(xp, slot_cap, tok_cols, x_off):
        # tok_cols: [DIM, l] feature-major tokens for this slot (l <= cap)
        run = x_off
        l = tok_cols.shape[1]
        for t0, tn in _halves(slot_cap):
            blk = np.zeros((KD, P, tn), dtype=_NP_MM)
            n = max(0, min(tn, l - t0))
            if n:
                blk[:, :, :n] = tok_cols[:, t0 : t0 + n].reshape(KD, P, n)
            xp[:, run : run + KD * tn] = blk.transpose(1, 0, 2).reshape(P, KD * tn)
            run += KD * tn

    in_maps = []
    for core in range(E):
        eA, sA, lA = slotsA[core]
        eB, sB, lB = slotsB[core]
        xp = np.zeros((P, KD * C), dtype=_NP_MM)
        gw = np.zeros((C,), dtype=np.float32)
        ixA = idx_lists[eA][sA : sA + lA]
        ixB = idx_lists[eB][sB : sB + lB]
        _pack_x(xp, cA, xt_mm[:, ixA], 0)
        _pack_x(xp, cB, xt_mm[:, ixB], KD * cA)
        if lA:
            gw[:lA] = gw_lists[eA][sA : sA + lA]
        if lB:
            gw[cA : cA + lB] = gw_lists[eB][sB : sB + lB]
        gwp = np.ascontiguousarray(np.broadcast_to(gw, (P, C)))
        w1a, w3a, w2a = pack(eA)
        w1b, w3b, w2b = pack(eB)
        in_maps.append(
            {
                "xp": xp,
                "gwp": gwp,
                "w1a": w1a,
                "w3a": w3a,
                "w2a": w2a,
                "w1b": w1b,
                "w3b": w3b,
                "w2b": w2b,
            }
        )

    res = run_bass_kernel_spmd(nc, in_maps, list(range(E)), trace=TRACE)
    global LAST_RESULTS
    LAST_RESULTS = res

    out = np.zeros((T, DIM), dtype=np.float32)
    for core in range(E):
        eA, sA, lA = slotsA[core]
        eB, sB, lB = slotsB[core]
        yt = res.results[core]["yt"].reshape(DIM, C)
        if lA:
            out[idx_lists[eA][sA : sA + lA]] += yt[:, :lA].T
        if lB:
            out[idx_lists[eB][sB : sB + lB]] += yt[:, cA : cA + lB].T
    return out.reshape(Bsz, Ssz, DIM)



# revision 17
# speedup vs baseline: 1.0146x; 1.0146x over previous
"""MoE (top-2 of 8 experts, SwiGLU) Trainium2 kernel — balanced 2-slot version.

Sharding strategy (expert-parallel with load balancing):
  - Host computes the gate (tiny [T,8] matmul), top-2 routing and softmax
    weights. Tokens are grouped by expert; each expert's token list is split
    across up to two fixed-capacity "slots". Every core runs the SAME program
    with two slots (capacities cA, cB chosen from the routing): each slot has
    its own expert weight set (W1/W3/W2) and token block, so per-core work is
    identical (cA+cB streamed columns) regardless of expert load imbalance.
  - Core: per slot, y = gate_w * (silu(x @ W1e.T) * (x @ W3e.T)) @ W2e.T in
    feature-major layout (features on partitions, tokens on the free axis).
  - Host scatter-adds each slot's output rows back into the full output.

Matmuls run in fp16 (full PE rate, fp32 PSUM accumulation). W1/W3 weight
streams, the gate vector and the y output ride the sync HWDGE queue; x and W2
ride the scalar HWDGE queue, so the PE is never starved during the ramp.
"""

import numpy as np

import concourse.bass as bass
import concourse.mybir as mybir
from concourse import bacc
from concourse import tile
from concourse.bass_utils import run_bass_kernel_spmd

DIM = 1024
HID = 2816
E = 8
TOPK = 2
P = 128
KD = DIM // P  # 8 k-tiles over DIM
KH = HID // P  # 22 k-tiles over HID
F32 = mybir.dt.float32
import os as _os
_MM_DT_NAME = _os.environ.get("KERNEL_MM_DT", "float16")
MM_DT = getattr(mybir.dt, _MM_DT_NAME)
_NP_MM = {"float32r": np.float32, "float32": np.float32}.get(_MM_DT_NAME)
if _NP_MM is None:
    import ml_dtypes as _mld
    _NP_MM = {"float16": np.float16, "bfloat16": _mld.bfloat16}[_MM_DT_NAME]
PREFETCH_W = 5  # weight h-tile pairs in flight (= wload bufs)

# Test hooks: when TRACE is set (by test.py), the SPMD launch captures an
# NTFF profile and the BassKernelResults lands in LAST_RESULTS.
TRACE = False
LAST_RESULTS = None

_nc_cache: dict = {}


def _halves(c):
    """Slot slicing: one slice if it fits a PSUM bank (512 fp32) — pipelining
    then comes from bufs=2 on the PSUM tag — else two halves."""
    if c <= 512:
        return [(0, c)]
    h = (c + 1) // 2
    return [(0, h), (h, c - h)]


def _build_nc(cA, cB):
    """Per-core Bass program: two expert slots of capacities cA and cB."""
    nc = bacc.Bacc(
        "TRN2",
        target_bir_lowering=False,
        debug=False,
        enable_asserts=False,
        num_devices=E,
    )

    C = cA + cB
    slot_caps = [cA, cB]
    slot_offs = [0, cA]
    slot_slices = [_halves(cA), _halves(cB)]
    # x regions: one contiguous [P, KD*tn] block per (slot, slice) so each is
    # a single DMA with multi-KB per-partition lines (fast, order-robust).
    x_offs = {}
    run = 0
    for s in range(2):
        for t0, tn in slot_slices[s]:
            x_offs[(s, t0)] = run
            run += KD * tn
    assert run == KD * C

    xt_d = nc.dram_tensor("xp", [P, KD * C], MM_DT, kind="ExternalInput").ap()
    w13_d = {}
    w2_d = {}
    for s, tag in enumerate("ab"):
        w13_d[s] = (
            nc.dram_tensor(f"w1{tag}", [KH, P, KD * P], MM_DT, kind="ExternalInput").ap(),
            nc.dram_tensor(f"w3{tag}", [KH, P, KD * P], MM_DT, kind="ExternalInput").ap(),
        )
        w2_d[s] = nc.dram_tensor(f"w2{tag}", [KD, P, KH * P], MM_DT, kind="ExternalInput").ap()
    gw_d = nc.dram_tensor("gwp", [P, C], F32, kind="ExternalInput").ap()
    yt_d = nc.dram_tensor("yt", [KD, P, C], MM_DT, kind="ExternalOutput").ap()

    with tile.TileContext(nc) as tc:
        with (
            tc.tile_pool(name="xpool", bufs=1) as xpool,
            tc.tile_pool(name="wload", bufs=PREFETCH_W) as wload,
            tc.tile_pool(name="w2load", bufs=3) as w2load,
            tc.tile_pool(name="gpool", bufs=1) as gpool,
            tc.tile_pool(name="spool", bufs=2) as spool,
            tc.tile_pool(name="ypool", bufs=3) as ypool,
            tc.tile_pool(name="psA", bufs=1, space="PSUM") as psApool,
            tc.tile_pool(name="psB", bufs=1, space="PSUM") as psBpool,
        ):
            # PE pre-warm: dummy matmuls on a zeroed tile keep the PE busy
            # (and flip the HAM clock gate to 8/8) while input DMAs stream.
            t_warm = xpool.tile([P, 512], mybir.dt.bfloat16, tag="warm")
            nc.vector.memset(t_warm, 0.0)
            ps_warm = psApool.tile([P, 512], F32, tag="ps_s0_0")
            for _ in range(11):
                nc.tensor.matmul(
                    ps_warm, lhsT=t_warm[:, :P], rhs=t_warm, start=True, stop=True
                )

            from collections import deque

            # ---- weight h-tile stream (sync queue, consumption order) ----
            w1_tiles: deque = deque()
            w3_tiles: deque = deque()
            wq = [(s, i) for s in range(2) for i in range(KH)]
            wq_pos = [0]

            def pump_w13():
                if wq_pos[0] >= len(wq):
                    return
                s, i = wq[wq_pos[0]]
                wq_pos[0] += 1
                halved = s == 0 and i < 2  # fine-grained ramp: stutter, don't
                # block long enough for the HAM clock gate to re-throttle
                for w13_idx, tag in ((0, "w1"), (1, "w3")):
                    t = wload.tile(
                        [P, KD * P], MM_DT, tag=tag, name=f"{tag}_{s}_{i}"
                    )
                    if halved:
                        h = (KD // 2) * P
                        nc.sync.dma_start(out=t[:, :h], in_=w13_d[s][w13_idx][i][:, :h])
                        nc.sync.dma_start(out=t[:, h:], in_=w13_d[s][w13_idx][i][:, h:])
                    else:
                        nc.sync.dma_start(out=t, in_=w13_d[s][w13_idx][i])
                    (w1_tiles if w13_idx == 0 else w3_tiles).append(t)

            # ---- w2 d-tile stream (scalar queue) ----
            w2_tiles: deque = deque()
            w2q = [(s, d) for s in range(2) for d in range(KD)]
            w2q_pos = [0]

            def pump_w2():
                if w2q_pos[0] >= len(w2q):
                    return
                s, dd = w2q[w2q_pos[0]]
                w2q_pos[0] += 1
                t2 = w2load.tile([P, KH * P], MM_DT, tag="w2", name=f"w2_{s}_{dd}")
                nc.scalar.dma_start(out=t2, in_=w2_d[s][dd])
                w2_tiles.append(t2)

            # Activations on the scalar queue (parallel with the sync-queue
            # weight stream): one contiguous DMA per (slot, slice), slot-a
            # slice-0 first — it gates the first real matmul.
            t_x = {}
            for s in range(2):
                for t0, tn in slot_slices[s]:
                    o = x_offs[(s, t0)]
                    t = xpool.tile(
                        [P, KD * tn], MM_DT, tag=f"x{s}_{t0}", name=f"x_{s}_{t0}"
                    )
                    if s == 0 and t0 == 0:
                        # first slice in two halves so the first k-tiles land
                        # (and unblock the first matmul group) sooner
                        h = (KD // 2) * tn
                        nc.scalar.dma_start(out=t[:, :h], in_=xt_d[:, o : o + h])
                        nc.scalar.dma_start(
                            out=t[:, h:], in_=xt_d[:, o + h : o + KD * tn]
                        )
                    else:
                        nc.scalar.dma_start(out=t, in_=xt_d[:, o : o + KD * tn])
                    t_x[(s, t0)] = t

            for _ in range(PREFETCH_W):
                pump_w13()  # (a, 0..4): (a, 0) is the ramp critical path
            t_gw = xpool.tile([P, C], F32, tag="gw")

            g_tiles = {0: [], 1: []}

            def stage1(s):
                cap = slot_caps[s]
                slices = slot_slices[s]
                for i in range(KH):
                    pump_w13()
                    if s == 0 and i == 2:
                        # gate weights ride the sync queue mid-stream; they are
                        # only needed by stage 2.
                        nc.sync.dma_start(out=t_gw, in_=gw_d)
                    if i == KH - 6:
                        pump_w2()  # this slot's first two w2 d-tiles load
                    if i == KH - 3:
                        pump_w2()  # during the stage-1 tail
                    t_w1 = w1_tiles.popleft()
                    t_w3 = w3_tiles.popleft()

                    t_g = gpool.tile([P, cap], MM_DT, tag=f"g{s}_{i}")
                    g_tiles[s].append(t_g)

                    nslice = len(slices)
                    ps1s, ps3s = [], []
                    for si, (t0, tn) in enumerate(slices):
                        ps1s.append(psApool.tile(
                            [P, 512], F32, tag=f"ps_s{s}_{si}",
                            bufs=(2 if nslice == 1 else 1), name=f"ps1_{s}_{i}_{t0}"
                        ))
                        ps3s.append(psBpool.tile(
                            [P, 512], F32, tag=f"ps_s{s}_{si}",
                            bufs=(2 if nslice == 1 else 1), name=f"ps3_{s}_{i}_{t0}"
                        ))
                    # Slices interleaved inside the k-loop: back-to-back
                    # matmuls share each weight tile (536 streamed cols per
                    # load instead of 268). The first two h-tiles keep the
                    # sequential order so the ramp only waits on slice 0's x.
                    if s == 0 and i < 2 and nslice > 1:
                        slice_phases = [[si] for si in range(nslice)]
                    else:
                        slice_phases = [list(range(nslice))]
                    for w_tile, ps_grp in ((t_w1, ps1s), (t_w3, ps3s)):
                        for phase in slice_phases:
                            for k in range(KD):
                                for si in phase:
                                    t0, tn = slices[si]
                                    nc.tensor.matmul(
                                        ps_grp[si][:, :tn],
                                        lhsT=w_tile[:, k * P : (k + 1) * P],
                                        rhs=t_x[(s, t0)][:, k * tn : (k + 1) * tn],
                                        start=(k == 0),
                                        stop=(k == KD - 1),
                                    )
                    for si, (t0, tn) in enumerate(slices):
                        t_sg = spool.tile([P, 512], F32, tag="sig")
                        nc.scalar.activation(
                            t_sg[:, :tn],
                            ps1s[si][:, :tn],
                            mybir.ActivationFunctionType.Sigmoid,
                        )
                        t_s = spool.tile([P, 512], F32, tag="silu")
                        nc.vector.tensor_mul(t_s[:, :tn], t_sg[:, :tn], ps1s[si][:, :tn])
                        nc.vector.tensor_mul(
                            t_g[:, t0 : t0 + tn], t_s[:, :tn], ps3s[si][:, :tn]
                        )

            def stage2(s):
                off = slot_offs[s]
                for dt_i in range(KD):
                    slices = slot_slices[s]
                    if s == 1 and dt_i == KD - 1 and len(slices) == 1:
                        # split the very last d-tile so the final mul + y DMA
                        # chain is half-size and overlaps the preceding MMs
                        c = slot_caps[s]
                        h = (c + 1) // 2
                        slices = [(0, h), (h, c - h)]
                    pump_w2()
                    t_w2 = w2_tiles.popleft()
                    nslice = len(slot_slices[s])
                    for si, (t0, tn) in enumerate(slices):
                        # a split-in-two last d-tile reuses tag 0 (bufs=2 ring)
                        tag_si = min(si, nslice - 1)
                        psy = psApool.tile(
                            [P, 512], F32, tag=f"ps_s{s}_{tag_si}",
                            bufs=(2 if nslice == 1 else 1), name=f"psy_{s}_{dt_i}_{t0}"
                        )
                        for i in range(KH):
                            nc.tensor.matmul(
                                psy[:, :tn],
                                lhsT=t_w2[:, i * P : (i + 1) * P],
                                rhs=g_tiles[s][i][:, t0 : t0 + tn],
                                start=(i == 0),
                                stop=(i == KH - 1),
                            )
                        t_y = ypool.tile([P, 512], MM_DT, tag="y")
                        nc.vector.tensor_mul(
                            t_y[:, :tn], psy[:, :tn], t_gw[:, off + t0 : off + t0 + tn]
                        )
                        nc.sync.dma_start(
                            out=yt_d[dt_i][:, off + t0 : off + t0 + tn],
                            in_=t_y[:, :tn],
                        )

            stage1(0)
            stage2(0)
            stage1(1)
            stage2(1)

    nc.compile()
    return nc


def _route(xt, Wg):
    """Top-2 routing identical to the reference (argmax twice + softmax)."""
    scores = xt @ Wg.T  # [T, E] fp32
    top1 = np.argmax(scores, axis=1)
    v1 = scores[np.arange(scores.shape[0]), top1]
    masked = scores.copy()
    masked[np.arange(scores.shape[0]), top1] = -np.inf
    top2 = np.argmax(masked, axis=1)
    v2 = masked[np.arange(scores.shape[0]), top2]
    # softmax over [v1, v2] in fp32 (v1 >= v2)
    e2 = np.exp((v2 - v1).astype(np.float32))
    w1 = (1.0 / (1.0 + e2)).astype(np.float32)
    w2 = (e2 / (1.0 + e2)).astype(np.float32)
    return top1, top2, w1, w2


def _plan_slots(counts):
    """Choose slot capacities (cA, cB) and assign each expert's tokens to
    A/B slots (8 of each, one per core) so every expert fits and cA+cB is
    minimal. Returns (cA, cB, slotsA, slotsB) with slots* = [(e, start, len)].
    """
    counts = np.asarray(counts, dtype=np.int64)
    order = np.argsort(-counts, kind="stable")
    n = counts[order]
    total = int(counts.sum())
    mean_cap = -(-total // E)
    best = None
    for k in range(0, E // 2 + 1):
        cA_min = -(-int(n[0]) // 2) if k > 0 else 0
        cB_min = -(-int(n[E - k]) // 2) if k > 0 else 0
        mid = n[k : E - k]
        mid_max = int(mid[0]) if len(mid) else 0
        Ck = max(cA_min + cB_min, mid_max, mean_cap)
        if best is None or Ck < best[0]:
            best = (Ck, k)
    Ck, k = best
    if k > 0:
        cB = -(-int(n[E - k]) // 2)
        cA = Ck - cB
    else:
        cA = (Ck + 1) // 2
        cB = Ck - cA
    # degenerate-routing guard: keep both slots non-trivial
    cA, cB = max(cA, 16), max(cB, 16)
    slotsA, slotsB = [], []
    for i, e in enumerate(order):
        ne = int(counts[e])
        if k > 0 and i < k:  # two A slots
            a1 = min(cA, ne)
            slotsA.append((e, 0, a1))
            slotsA.append((e, a1, ne - a1))
        elif i >= E - k:  # two B slots
            b1 = min(cB, ne)
            slotsB.append((e, 0, b1))
            slotsB.append((e, b1, ne - b1))
        else:  # one A + one B slot
            a1 = min(cA, ne)
            slotsA.append((e, 0, a1))
            slotsB.append((e, a1, ne - a1))
    assert len(slotsA) == E and len(slotsB) == E
    assert all(l <= cA for _, _, l in slotsA)
    assert all(l <= cB for _, _, l in slotsB)
    return cA, cB, slotsA, slotsB


def _pack_weights(W1, W3, W2, e):
    # w1p[i, p, k, c] = W1T[k*P+p, i*P+c] = W1[e, i*P+c, k*P+p]
    w1p = np.ascontiguousarray(
        W1[e].reshape(KH, P, KD, P).transpose(0, 3, 2, 1).astype(_NP_MM)
    ).reshape(KH, P, KD * P)
    w3p = np.ascontiguousarray(
        W3[e].reshape(KH, P, KD, P).transpose(0, 3, 2, 1).astype(_NP_MM)
    ).reshape(KH, P, KD * P)
    # w2p[dt, p, i, c] = W2T[i*P+p, dt*P+c] = W2[e, dt*P+c, i*P+p]
    w2p = np.ascontiguousarray(
        W2[e].reshape(KD, P, KH, P).transpose(0, 3, 2, 1).astype(_NP_MM)
    ).reshape(KD, P, KH * P)
    return w1p, w3p, w2p


def kernel(x, Wg, W1, W3, W2):
    x = np.asarray(x, dtype=np.float32)
    Wg = np.asarray(Wg, dtype=np.float32)
    W1 = np.asarray(W1, dtype=np.float32)
    W3 = np.asarray(W3, dtype=np.float32)
    W2 = np.asarray(W2, dtype=np.float32)

    Bsz, Ssz, _ = x.shape
    T = Bsz * Ssz
    xt = x.reshape(T, DIM)

    top1, top2, wt1, wt2 = _route(xt, Wg)

    idx_lists = []
    gw_lists = []
    counts = np.zeros(E, dtype=np.int64)
    for e in range(E):
        m1 = np.nonzero(top1 == e)[0]
        m2 = np.nonzero(top2 == e)[0]
        idx_lists.append(np.concatenate([m1, m2]))
        gw_lists.append(np.concatenate([wt1[m1], wt2[m2]]))
        counts[e] = len(idx_lists[e])

    cA, cB, slotsA, slotsB = _plan_slots(counts)
    C = cA + cB

    if (cA, cB) not in _nc_cache:
        _nc_cache[(cA, cB)] = _build_nc(cA, cB)
    nc = _nc_cache[(cA, cB)]

    packs = {e: None for e in range(E)}

    def pack(e):
        if packs[e] is None:
            packs[e] = _pack_weights(W1, W3, W2, e)
        return packs[e]

    xt_mm = xt.T.astype(_NP_MM)  # [DIM, T] once

    # x region layout must match _build_nc: [(slot, slice)] contiguous blocks
    # of [P, KD*tn], per-partition line = the KD k-tiles of that slice.
    def _pack_x(xp, slot_cap, tok_cols, x_off):
        # tok_cols: [DIM, l] feature-major tokens for this slot (l <= cap)
        run = x_off
        l = tok_cols.shape[1]
        for t0, tn in _halves(slot_cap):
            blk = np.zeros((KD, P, tn), dtype=_NP_MM)
            n = max(0, min(tn, l - t0))
            if n:
                blk[:, :, :n] = tok_cols[:, t0 : t0 + n].reshape(KD, P, n)
            xp[:, run : run + KD * tn] = blk.transpose(1, 0, 2).reshape(P, KD * tn)
            run += KD * tn

    in_maps = []
    for core in range(E):
        eA, sA, lA = slotsA[core]
        eB, sB, lB = slotsB[core]
        xp = np.zeros((P, KD * C), dtype=_NP_MM)
        gw = np.zeros((C,), dtype=np.float32)
        ixA = idx_lists[eA][sA : sA + lA]
        ixB = idx_lists[eB][sB : sB + lB]
        _pack_x(xp, cA, xt_mm[:, ixA], 0)
        _pack_x(xp, cB, xt_mm[:, ixB], KD * cA)
        if lA:
            gw[:lA] = gw_lists[eA][sA : sA + lA]
        if lB:
            gw[cA : cA + lB] = gw_lists[eB][sB : sB + lB]
        gwp = np.ascontiguousarray(np.broadcast_to(gw, (P, C)))
        w1a, w3a, w2a = pack(eA)
        w1b, w3b, w2b = pack(eB)
        in_maps.append(
            {
                "xp": xp,
                "gwp": gwp,
                "w1a": w1a,
                "w3a": w3a,
                "w2a": w2a,
                "w1b": w1b,
                "w3b": w3b,
                "w2b": w2b,
            }
        )

    res = run_bass_kernel_spmd(nc, in_maps, list(range(E)), trace=TRACE)
    global LAST_RESULTS
    LAST_RESULTS = res

    out = np.zeros((T, DIM), dtype=np.float32)
    for core in range(E):
        eA, sA, lA = slotsA[core]
        eB, sB, lB = slotsB[core]
        yt = res.results[core]["yt"].reshape(DIM, C)
        if lA:
            out[idx_lists[eA][sA : sA + lA]] += yt[:, :lA].T
        if lB:
            out[idx_lists[eB][sB : sB + lB]] += yt[:, cA : cA + lB].T
    return out.reshape(Bsz, Ssz, DIM)

